# revision 6
# baseline (speedup 1.0000x reference)
"""nn_PhaseAwareAttention kernel for 8 Trainium2 NeuronCores.

Algebraic collapse: softmax over a size-1 axis is identically 1, so the
q/k branch (and both node gathers) never affect the output:

    out = edge_attr + 0.5*(((edge_attr @ Wv.T + bv) @ Wiv.T + biv) @ W_mo.T
                           + b_mo) @ Wo.T + bo
        = edge_attr @ (I + M) + c,   M = 0.5*(Wo @ W_mo @ Wiv @ Wv).T

so the device kernel is a single streamed 128x128 matmul over the edge
axis, sharded across 8 cores with edge_attr transposed to [HID, E/8]
(contraction dim on partitions).

The kernel is fabric/HBM bound (~427 GB/s/core aggregate), so the win
is moving fewer bytes:
  MODE "bf16": y = x@(I+M) fully on device, bf16 in / bf16 out (16 MB).
  MODE "fp8" : device computes only the correction c = x@(64*M) from an
      fp8 input and returns it as fp8 (8 MB total); the residual
      out = x + c/64 is reconstructed on the host during unsharding.
      fp8 quantization errors only touch the ~5%-magnitude correction
      term, giving ~3e-3 relative error against the 2e-2 tolerance.

Engine layout: SP ring streams x in; GpSimd (SWDGE ring) streams y out;
PE runs 512-col matmuls into [128,2048] PSUM megatiles (2 x 4 banks,
double buffered); the PSUM drain is split DVE (tensor_copy) / ACT
(activation Copy) so no single engine paces the pipeline (DVE alone at
~123 G elem/s would).
"""

import numpy as np
import ml_dtypes

import concourse.bacc as bacc
import concourse.mybir as mybir
from concourse.bass_utils import run_bass_kernel_spmd
from concourse.tile import TileContext

E = 250000
HID = 128
NCORES = 8
ESH = E // NCORES          # 31250 edges per core
BIG = 8192                 # max edges per input DMA chunk
OUT = 4096                 # edges per output DMA
MEGA = 2048                # edges per PSUM megatile (4 fp32 banks)
SUB = 512                  # edges per matmul (one PSUM bank of fp32)
# DVE processes 0.96 elem/ns/partition, ACT 1.2 -> split each megatile
# ~46/54 so both drain engines finish together.
DVE_COLS = 936
# Taper both ends: small first chunks start the pipe early, small last
# chunks shorten the drain/store tail.
CHUNKS = [1024, 2048, 4096, 8192, 8192, 4096, 2048, 1024, 530]
assert sum(CHUNKS) == ESH
assert all(c <= BIG for c in CHUNKS)

MODE = "fp8"               # "bf16" | "fp8"
FP8_WSCALE = 64.0          # keeps 64*M and 64*c in fp8 e4m3 normal range

BF16 = ml_dtypes.bfloat16
FP8 = ml_dtypes.float8_e4m3   # TRN FP8_EXP4 semantics (max normal 240)

_PROGRAM_CACHE = {}


def _build_program(mode):
    key = ("nc", mode)
    if key in _PROGRAM_CACHE:
        return _PROGRAM_CACHE[key]

    nc = bacc.Bacc()
    f32 = mybir.dt.float32
    dt = mybir.dt.bfloat16 if mode == "bf16" else mybir.dt.float8e4
    copy_fn = mybir.ActivationFunctionType.Copy

    xt = nc.dram_tensor("xt", [HID, ESH], dt, kind="ExternalInput")
    wm = nc.dram_tensor("wm", [HID, HID], dt, kind="ExternalInput")
    yt = nc.dram_tensor("yt", [HID, ESH], dt, kind="ExternalOutput")

    with TileContext(nc) as tc:
        with (
            tc.tile_pool(name="const", bufs=1) as cpool,
            tc.tile_pool(name="xraw", bufs=3) as rpool,
            tc.tile_pool(name="yout", bufs=4) as opool,
            tc.tile_pool(name="psum", bufs=2, space="PSUM") as ppool,
        ):
            w_tile = cpool.tile([HID, HID], dt)
            nc.sync.dma_start(out=w_tile, in_=wm[:, :])

            c0 = 0
            for cw in CHUNKS:
                x_raw = rpool.tile([HID, BIG], dt)
                nc.sync.dma_start(out=x_raw[:, :cw], in_=xt[:, c0 : c0 + cw])
                for u0 in range(0, cw, OUT):
                    uw = min(OUT, cw - u0)
                    o_tile = opool.tile([HID, OUT], dt)
                    for m0 in range(0, uw, MEGA):
                        mw = min(MEGA, uw - m0)
                        ps = ppool.tile([HID, MEGA], f32)
                        for s in range(0, mw, SUB):
                            n = min(SUB, mw - s)
                            nc.tensor.matmul(
                                ps[:, s : s + n], w_tile,
                                x_raw[:, u0 + m0 + s : u0 + m0 + s + n],
                                start=True, stop=True,
                            )
                        dv = min(int(round(mw * DVE_COLS / MEGA)), mw)
                        if dv:
                            nc.vector.tensor_copy(
                                o_tile[:, m0 : m0 + dv], ps[:, :dv]
                            )
                        if mw - dv:
                            nc.scalar.activation(
                                o_tile[:, m0 + dv : m0 + mw], ps[:, dv:mw],
                                copy_fn,
                            )
                    nc.gpsimd.dma_start(
                        out=yt[:, c0 + u0 : c0 + u0 + uw], in_=o_tile[:, :uw]
                    )
                c0 += cw

    nc.finalize()
    _PROGRAM_CACHE[key] = nc
    return nc


def _prepare(inputs):
    x = np.ascontiguousarray(inputs["edge_attr"], dtype=np.float32)

    Wv = inputs["Wv"].astype(np.float64)
    bv = inputs["bv"].astype(np.float64)
    W_in = inputs["W_in"].astype(np.float64)
    b_in = inputs["b_in"].astype(np.float64)
    Wiv = W_in[2 * HID : 3 * HID]
    biv = b_in[2 * HID : 3 * HID]
    W_mo = inputs["W_mo"].astype(np.float64)
    b_mo = inputs["b_mo"].astype(np.float64)
    Wo = inputs["Wo"].astype(np.float64)
    bo = inputs["bo"].astype(np.float64)

    M = 0.5 * (Wo @ W_mo @ Wiv @ Wv).T
    c = 0.5 * (((bv @ Wiv.T + biv) @ W_mo.T + b_mo) @ Wo.T + bo)

    if MODE == "bf16":
        wdev = np.ascontiguousarray(np.eye(HID) + M).astype(BF16)
        xdt = BF16
    else:
        wdev = np.ascontiguousarray(FP8_WSCALE * M).astype(FP8)
        xdt = FP8

    cf = c.astype(np.float32)

    nc = _build_program(MODE)

    in_maps = []
    for i in range(NCORES):
        shard = x[i * ESH : (i + 1) * ESH]
        in_maps.append(
            {"xt": np.ascontiguousarray(shard.T).astype(xdt), "wm": wdev}
        )

    return nc, in_maps, cf


def kernel(**inputs) -> np.ndarray:
    nc, in_maps, cf = _prepare(inputs)

    res = run_bass_kernel_spmd(nc, in_maps, list(range(NCORES)))

    out = np.empty((E, HID), dtype=np.float32)
    for i in range(NCORES):
        y = res.results[i]["yt"].astype(np.float32).T
        if MODE == "fp8":
            sh = inputs["edge_attr"][i * ESH : (i + 1) * ESH]
            out[i * ESH : (i + 1) * ESH] = sh + y * (1.0 / FP8_WSCALE)
        else:
            out[i * ESH : (i + 1) * ESH] = y
    if np.any(cf != 0.0):
        out += cf[None, :]
    return out


# revision 8
# speedup vs baseline: 1.0528x; 1.0528x over previous
"""nn_PhaseAwareAttention kernel for 8 Trainium2 NeuronCores.

Algebraic collapse: softmax over a size-1 axis is identically 1, so the
q/k branch (and both node gathers) never affect the output:

    out = edge_attr + 0.5*(((edge_attr @ Wv.T + bv) @ Wiv.T + biv) @ W_mo.T
                           + b_mo) @ Wo.T + bo
        = edge_attr @ (I + M) + c,   M = 0.5*(Wo @ W_mo @ Wiv @ Wv).T

so the device kernel is a single streamed 128x128 matmul over the edge
axis, sharded across 8 cores with edge_attr transposed to [HID, E/8]
(contraction dim on partitions).

The kernel is fabric/HBM bound (~427 GB/s/core aggregate), so the win
is moving fewer bytes:
  MODE "bf16": y = x@(I+M) fully on device, bf16 in / bf16 out (16 MB).
  MODE "fp8" : device computes only the correction c = x@(64*M) from an
      fp8 input and returns it as fp8 (8 MB total); the residual
      out = x + c/64 is reconstructed on the host during unsharding.
      fp8 quantization errors only touch the ~5%-magnitude correction
      term, giving ~3e-3 relative error against the 2e-2 tolerance.

Engine layout: SP ring streams x in; GpSimd (SWDGE ring) streams y out;
PE runs 512-col matmuls into [128,2048] PSUM megatiles (2 x 4 banks,
double buffered); the PSUM drain is split DVE (tensor_copy) / ACT
(activation Copy) so no single engine paces the pipeline (DVE alone at
~123 G elem/s would).
"""

import numpy as np
import ml_dtypes

import concourse.bacc as bacc
import concourse.mybir as mybir
from concourse.bass_utils import run_bass_kernel_spmd
from concourse.tile import TileContext

E = 250000
HID = 128
NCORES = 8
ESH = E // NCORES          # 31250 edges per core
BIG = 8192                 # max edges per input DMA chunk
MEGA = 2048                # edges per PSUM megatile (4 fp32 banks)
SUB = 512                  # edges per matmul (one PSUM bank of fp32)
# Taper both ends: small first chunks start the pipe early, small last
# chunks shorten the drain/store tail.
CHUNKS = [1024, 2048, 4096, 8192, 8192, 4096, 2048, 1024, 530]
assert sum(CHUNKS) == ESH
assert all(c <= BIG for c in CHUNKS)
# Per-column drain cost (ns) on each engine, plus fixed per-instruction
# overhead (PSUM access bubble + sequencer) used to load-balance whole
# megatiles across the two PSUM-capable engines.
DVE_NS = (1.042, 170.0)
ACT_NS = (0.833, 210.0)

MODE = "fp8"               # "bf16" | "fp8"
FP8_WSCALE = 64.0          # keeps 64*M and 64*c in fp8 e4m3 normal range

BF16 = ml_dtypes.bfloat16
FP8 = ml_dtypes.float8_e4m3   # TRN FP8_EXP4 semantics (max normal 240)

_PROGRAM_CACHE = {}


def _build_program(mode):
    key = ("nc", mode)
    if key in _PROGRAM_CACHE:
        return _PROGRAM_CACHE[key]

    nc = bacc.Bacc()
    f32 = mybir.dt.float32
    dt = mybir.dt.bfloat16 if mode == "bf16" else mybir.dt.float8e4
    copy_fn = mybir.ActivationFunctionType.Copy

    xt = nc.dram_tensor("xt", [HID, ESH], dt, kind="ExternalInput")
    wm = nc.dram_tensor("wm", [HID, HID], dt, kind="ExternalInput")
    yt = nc.dram_tensor("yt", [HID, ESH], dt, kind="ExternalOutput")

    with TileContext(nc) as tc:
        with (
            tc.tile_pool(name="const", bufs=1) as cpool,
            tc.tile_pool(name="xraw", bufs=3) as rpool,
            tc.tile_pool(name="yout", bufs=6) as opool,
            tc.tile_pool(name="psum", bufs=2, space="PSUM") as ppool,
        ):
            w_tile = cpool.tile([HID, HID], dt)
            nc.sync.dma_start(out=w_tile, in_=wm[:, :])

            # Each megatile is drained by exactly ONE engine (no shared
            # PSUM/o_tile between DVE and ACT -> far fewer cross-engine
            # semaphores); whole megatiles are load-balanced by the
            # engines' measured per-column copy rates.
            eng_load = [0.0, 0.0]  # DVE, ACT accumulated ns
            c0 = 0
            for cw in CHUNKS:
                x_raw = rpool.tile([HID, BIG], dt)
                nc.sync.dma_start(out=x_raw[:, :cw], in_=xt[:, c0 : c0 + cw])
                for m0 in range(0, cw, MEGA):
                    mw = min(MEGA, cw - m0)
                    ps = ppool.tile([HID, MEGA], f32)
                    for s in range(0, mw, SUB):
                        n = min(SUB, mw - s)
                        nc.tensor.matmul(
                            ps[:, s : s + n], w_tile,
                            x_raw[:, m0 + s : m0 + s + n],
                            start=True, stop=True,
                        )
                    o_tile = opool.tile([HID, MEGA], dt)
                    cost = [mw * DVE_NS[0] + DVE_NS[1],
                            mw * ACT_NS[0] + ACT_NS[1]]
                    pick = 0 if eng_load[0] + cost[0] <= eng_load[1] + cost[1] else 1
                    eng_load[pick] += cost[pick]
                    if pick == 0:
                        nc.vector.tensor_copy(o_tile[:, :mw], ps[:, :mw])
                    else:
                        nc.scalar.activation(o_tile[:, :mw], ps[:, :mw], copy_fn)
                    nc.gpsimd.dma_start(
                        out=yt[:, c0 + m0 : c0 + m0 + mw], in_=o_tile[:, :mw]
                    )
                c0 += cw

    nc.finalize()
    _PROGRAM_CACHE[key] = nc
    return nc


def _prepare(inputs):
    x = np.ascontiguousarray(inputs["edge_attr"], dtype=np.float32)

    Wv = inputs["Wv"].astype(np.float64)
    bv = inputs["bv"].astype(np.float64)
    W_in = inputs["W_in"].astype(np.float64)
    b_in = inputs["b_in"].astype(np.float64)
    Wiv = W_in[2 * HID : 3 * HID]
    biv = b_in[2 * HID : 3 * HID]
    W_mo = inputs["W_mo"].astype(np.float64)
    b_mo = inputs["b_mo"].astype(np.float64)
    Wo = inputs["Wo"].astype(np.float64)
    bo = inputs["bo"].astype(np.float64)

    M = 0.5 * (Wo @ W_mo @ Wiv @ Wv).T
    c = 0.5 * (((bv @ Wiv.T + biv) @ W_mo.T + b_mo) @ Wo.T + bo)

    if MODE == "bf16":
        wdev = np.ascontiguousarray(np.eye(HID) + M).astype(BF16)
        xdt = BF16
    else:
        wdev = np.ascontiguousarray(FP8_WSCALE * M).astype(FP8)
        xdt = FP8

    cf = c.astype(np.float32)

    nc = _build_program(MODE)

    in_maps = []
    for i in range(NCORES):
        shard = x[i * ESH : (i + 1) * ESH]
        in_maps.append(
            {"xt": np.ascontiguousarray(shard.T).astype(xdt), "wm": wdev}
        )

    return nc, in_maps, cf


def kernel(**inputs) -> np.ndarray:
    nc, in_maps, cf = _prepare(inputs)

    res = run_bass_kernel_spmd(nc, in_maps, list(range(NCORES)))

    out = np.empty((E, HID), dtype=np.float32)
    for i in range(NCORES):
        y = res.results[i]["yt"].astype(np.float32).T
        if MODE == "fp8":
            sh = inputs["edge_attr"][i * ESH : (i + 1) * ESH]
            out[i * ESH : (i + 1) * ESH] = sh + y * (1.0 / FP8_WSCALE)
        else:
            out[i * ESH : (i + 1) * ESH] = y
    if np.any(cf != 0.0):
        out += cf[None, :]
    return out


# revision 10
# speedup vs baseline: 1.1299x; 1.0733x over previous
"""nn_PhaseAwareAttention kernel for 8 Trainium2 NeuronCores.

Algebraic collapse: softmax over a size-1 axis is identically 1, so the
q/k branch (and both node gathers) never affect the output:

    out = edge_attr + 0.5*(((edge_attr @ Wv.T + bv) @ Wiv.T + biv) @ W_mo.T
                           + b_mo) @ Wo.T + bo
        = edge_attr @ (I + M) + c,   M = 0.5*(Wo @ W_mo @ Wiv @ Wv).T

so the device kernel is a single streamed 128x128 matmul over the edge
axis, sharded across 8 cores with edge_attr transposed to [HID, E/8]
(contraction dim on partitions).

The kernel is fabric/HBM bound (~427 GB/s/core aggregate), so the win
is moving fewer bytes:
  MODE "bf16": y = x@(I+M) fully on device, bf16 in / bf16 out (16 MB).
  MODE "fp8" : device computes only the correction c = x@(64*M) from an
      fp8 input and returns it as fp8 (8 MB total); the residual
      out = x + c/64 is reconstructed on the host during unsharding.
      fp8 quantization errors only touch the ~5%-magnitude correction
      term, giving ~3e-3 relative error against the 2e-2 tolerance.

Engine layout: SP ring streams x in; GpSimd (SWDGE ring) streams y out;
PE runs 512-col matmuls into [128,2048] PSUM megatiles (2 x 4 banks,
double buffered); the PSUM drain is split DVE (tensor_copy) / ACT
(activation Copy) so no single engine paces the pipeline (DVE alone at
~123 G elem/s would).
"""

import numpy as np
import ml_dtypes

import concourse.bacc as bacc
import concourse.mybir as mybir
from concourse.bass_utils import run_bass_kernel_spmd
from concourse.tile import TileContext

E = 250000
HID = 128
NCORES = 8
ESH = E // NCORES          # 31250 edges per core
BIG = 8192                 # max edges per input DMA chunk
PAIR = 2048                # edges per output staging tile / output DMA
MEGA = 1024                # edges per PSUM tile (2 fp32 banks, 4 bufs)
SUB = 512                  # edges per matmul (one PSUM bank of fp32)
# Taper both ends: small first chunks start the pipe early, small last
# chunks shorten the drain/store tail.
CHUNKS = [1024, 2048, 4096, 8192, 8192, 4096, 2048, 1024, 530]
assert sum(CHUNKS) == ESH
assert all(c <= BIG for c in CHUNKS)
# Per-column drain cost (ns) on each engine, plus fixed per-instruction
# overhead (PSUM access bubble + sequencer) used to load-balance whole
# megatiles across the two PSUM-capable engines.
DVE_NS = (1.042, 170.0)
ACT_NS = (0.833, 210.0)

MODE = "fp8"               # "bf16" | "fp8"
FP8_WSCALE = 64.0          # keeps 64*M and 64*c in fp8 e4m3 normal range

BF16 = ml_dtypes.bfloat16
FP8 = ml_dtypes.float8_e4m3   # TRN FP8_EXP4 semantics (max normal 240)

_PROGRAM_CACHE = {}


def _build_program(mode):
    key = ("nc", mode)
    if key in _PROGRAM_CACHE:
        return _PROGRAM_CACHE[key]

    nc = bacc.Bacc()
    f32 = mybir.dt.float32
    dt = mybir.dt.bfloat16 if mode == "bf16" else mybir.dt.float8e4
    copy_fn = mybir.ActivationFunctionType.Copy

    xt = nc.dram_tensor("xt", [HID, ESH], dt, kind="ExternalInput")
    wm = nc.dram_tensor("wm", [HID, HID], dt, kind="ExternalInput")
    yt = nc.dram_tensor("yt", [HID, ESH], dt, kind="ExternalOutput")

    with TileContext(nc) as tc:
        with (
            tc.tile_pool(name="const", bufs=1) as cpool,
            tc.tile_pool(name="xraw", bufs=3) as rpool,
            tc.tile_pool(name="yout", bufs=6) as opool,
            tc.tile_pool(name="psum", bufs=4, space="PSUM") as ppool,
        ):
            w_tile = cpool.tile([HID, HID], dt)
            nc.sync.dma_start(out=w_tile, in_=wm[:, :])

            # 4 PSUM tiles of 2 banks each keep the per-buffer
            # MM -> drain -> MM recycle chain off the critical path.
            # Drains are assigned per PAIR of consecutive PSUM tiles so
            # each o_tile has a single writer (no cross-engine sems),
            # load-balanced by the engines' measured copy rates.
            eng_load = [0.0, 0.0]  # DVE, ACT accumulated ns
            c0 = 0
            for cw in CHUNKS:
                x_raw = rpool.tile([HID, BIG], dt)
                nc.sync.dma_start(out=x_raw[:, :cw], in_=xt[:, c0 : c0 + cw])
                for p0 in range(0, cw, PAIR):
                    pw = min(PAIR, cw - p0)
                    o_tile = opool.tile([HID, PAIR], dt)
                    n_inst = (pw + MEGA - 1) // MEGA
                    cost = [pw * DVE_NS[0] + n_inst * DVE_NS[1],
                            pw * ACT_NS[0] + n_inst * ACT_NS[1]]
                    pick = 0 if eng_load[0] + cost[0] <= eng_load[1] + cost[1] else 1
                    eng_load[pick] += cost[pick]
                    for m0 in range(p0, p0 + pw, MEGA):
                        mw = min(MEGA, p0 + pw - m0)
                        ps = ppool.tile([HID, MEGA], f32)
                        for s in range(0, mw, SUB):
                            n = min(SUB, mw - s)
                            nc.tensor.matmul(
                                ps[:, s : s + n], w_tile,
                                x_raw[:, m0 + s : m0 + s + n],
                                start=True, stop=True,
                            )
                        od = o_tile[:, m0 - p0 : m0 - p0 + mw]
                        if pick == 0:
                            nc.vector.tensor_copy(od, ps[:, :mw])
                        else:
                            nc.scalar.activation(od, ps[:, :mw], copy_fn)
                    nc.gpsimd.dma_start(
                        out=yt[:, c0 + p0 : c0 + p0 + pw], in_=o_tile[:, :pw]
                    )
                c0 += cw

    nc.finalize()
    _PROGRAM_CACHE[key] = nc
    return nc


def _prepare(inputs):
    x = np.ascontiguousarray(inputs["edge_attr"], dtype=np.float32)

    Wv = inputs["Wv"].astype(np.float64)
    bv = inputs["bv"].astype(np.float64)
    W_in = inputs["W_in"].astype(np.float64)
    b_in = inputs["b_in"].astype(np.float64)
    Wiv = W_in[2 * HID : 3 * HID]
    biv = b_in[2 * HID : 3 * HID]
    W_mo = inputs["W_mo"].astype(np.float64)
    b_mo = inputs["b_mo"].astype(np.float64)
    Wo = inputs["Wo"].astype(np.float64)
    bo = inputs["bo"].astype(np.float64)

    M = 0.5 * (Wo @ W_mo @ Wiv @ Wv).T
    c = 0.5 * (((bv @ Wiv.T + biv) @ W_mo.T + b_mo) @ Wo.T + bo)

    if MODE == "bf16":
        wdev = np.ascontiguousarray(np.eye(HID) + M).astype(BF16)
        xdt = BF16
    else:
        wdev = np.ascontiguousarray(FP8_WSCALE * M).astype(FP8)
        xdt = FP8

    cf = c.astype(np.float32)

    nc = _build_program(MODE)

    in_maps = []
    for i in range(NCORES):
        shard = x[i * ESH : (i + 1) * ESH]
        in_maps.append(
            {"xt": np.ascontiguousarray(shard.T).astype(xdt), "wm": wdev}
        )

    return nc, in_maps, cf


def kernel(**inputs) -> np.ndarray:
    nc, in_maps, cf = _prepare(inputs)

    res = run_bass_kernel_spmd(nc, in_maps, list(range(NCORES)))

    out = np.empty((E, HID), dtype=np.float32)
    for i in range(NCORES):
        y = res.results[i]["yt"].astype(np.float32).T
        if MODE == "fp8":
            sh = inputs["edge_attr"][i * ESH : (i + 1) * ESH]
            out[i * ESH : (i + 1) * ESH] = sh + y * (1.0 / FP8_WSCALE)
        else:
            out[i * ESH : (i + 1) * ESH] = y
    if np.any(cf != 0.0):
        out += cf[None, :]
    return out


# revision 15
# speedup vs baseline: 1.1761x; 1.0409x over previous
"""nn_PhaseAwareAttention kernel for 8 Trainium2 NeuronCores.

Algebraic collapse: softmax over a size-1 axis is identically 1, so the
q/k branch (and both node gathers) never affect the output:

    out = edge_attr + 0.5*(((edge_attr @ Wv.T + bv) @ Wiv.T + biv) @ W_mo.T
                           + b_mo) @ Wo.T + bo
        = edge_attr @ (I + M) + c,   M = 0.5*(Wo @ W_mo @ Wiv @ Wv).T

so the device kernel is a single streamed 128x128 matmul over the edge
axis, sharded across 8 cores with edge_attr transposed to [HID, E/8]
(contraction dim on partitions).

The kernel is fabric/HBM bound (~427 GB/s/core aggregate), so the win
is moving fewer bytes:
  MODE "bf16": y = x@(I+M) fully on device, bf16 in / bf16 out (16 MB).
  MODE "fp8" : device computes only the correction c = x@(64*M) from an
      fp8 input and returns it as fp8 (8 MB total); the residual
      out = x + c/64 is reconstructed on the host during unsharding.
      fp8 quantization errors only touch the ~5%-magnitude correction
      term, giving ~3e-3 relative error against the 2e-2 tolerance.

Engine layout: SP ring streams x in; GpSimd (SWDGE ring) streams y out;
PE runs 512-col matmuls into [128,2048] PSUM megatiles (2 x 4 banks,
double buffered); the PSUM drain is split DVE (tensor_copy) / ACT
(activation Copy) so no single engine paces the pipeline (DVE alone at
~123 G elem/s would).
"""

import numpy as np
import ml_dtypes

import concourse.bacc as bacc
import concourse.mybir as mybir
from concourse.bass_utils import run_bass_kernel_spmd
from concourse.tile import TileContext

E = 250000
HID = 128
NCORES = 8
ESH = E // NCORES          # 31250 edges per core
BIG = 4096                 # max edges per input DMA chunk
PAIR = 2048                # edges per output staging tile / output DMA
MEGA = 1024                # edges per PSUM tile (2 fp32 banks, 4 bufs)
SUB = 512                  # edges per matmul (one PSUM bank of fp32)
# Near-uniform chunks keep the pipeline stages rate-matched; small
# chunks at both ends start the pipe early and shorten the tail.
# The total DMA count (input chunks + output pairs + weight) must stay
# <= ~28: larger counts fail NEFF load (semaphore/ring exhaustion).
CHUNKS = [1024, 1024] + [4096] * 7 + [530]
assert sum(CHUNKS) == ESH
assert all(c <= BIG for c in CHUNKS)
# Per-column drain cost (ns) on each engine, plus fixed per-instruction
# overhead (PSUM access bubble + sequencer) used to load-balance whole
# megatiles across the two PSUM-capable engines.
DVE_NS = (1.042, 170.0)
ACT_NS = (0.833, 210.0)

MODE = "fp8"               # "bf16" | "fp8"
FP8_WSCALE = 64.0          # keeps 64*M and 64*c in fp8 e4m3 normal range

BF16 = ml_dtypes.bfloat16
FP8 = ml_dtypes.float8_e4m3   # TRN FP8_EXP4 semantics (max normal 240)

_PROGRAM_CACHE = {}


def _build_program(mode):
    key = ("nc", mode)
    if key in _PROGRAM_CACHE:
        return _PROGRAM_CACHE[key]

    nc = bacc.Bacc()
    f32 = mybir.dt.float32
    dt = mybir.dt.bfloat16 if mode == "bf16" else mybir.dt.float8e4
    copy_fn = mybir.ActivationFunctionType.Copy

    xt = nc.dram_tensor("xt", [HID, ESH], dt, kind="ExternalInput")
    wm = nc.dram_tensor("wm", [HID, HID], dt, kind="ExternalInput")
    yt = nc.dram_tensor("yt", [HID, ESH], dt, kind="ExternalOutput")

    with TileContext(nc) as tc:
        with (
            tc.tile_pool(name="const", bufs=1) as cpool,
            tc.tile_pool(name="xraw", bufs=4) as rpool,
            tc.tile_pool(name="yout", bufs=6) as opool,
            tc.tile_pool(name="psum", bufs=4, space="PSUM") as ppool,
        ):
            w_tile = cpool.tile([HID, HID], dt)
            nc.sync.dma_start(out=w_tile, in_=wm[:, :])

            # 4 PSUM tiles of 2 banks each keep the per-buffer
            # MM -> drain -> MM recycle chain off the critical path.
            # Drains are assigned per PAIR of consecutive PSUM tiles so
            # each o_tile has a single writer (no cross-engine sems),
            # load-balanced by the engines' measured copy rates.
            eng_load = [0.0, 0.0]  # DVE, ACT accumulated ns
            c0 = 0
            for cw in CHUNKS:
                x_raw = rpool.tile([HID, BIG], dt)
                nc.sync.dma_start(out=x_raw[:, :cw], in_=xt[:, c0 : c0 + cw])
                for p0 in range(0, cw, PAIR):
                    pw = min(PAIR, cw - p0)
                    o_tile = opool.tile([HID, PAIR], dt)
                    n_inst = (pw + MEGA - 1) // MEGA
                    cost = [pw * DVE_NS[0] + n_inst * DVE_NS[1],
                            pw * ACT_NS[0] + n_inst * ACT_NS[1]]
                    pick = 0 if eng_load[0] + cost[0] <= eng_load[1] + cost[1] else 1
                    eng_load[pick] += cost[pick]
                    for m0 in range(p0, p0 + pw, MEGA):
                        mw = min(MEGA, p0 + pw - m0)
                        ps = ppool.tile([HID, MEGA], f32)
                        for s in range(0, mw, SUB):
                            n = min(SUB, mw - s)
                            nc.tensor.matmul(
                                ps[:, s : s + n], w_tile,
                                x_raw[:, m0 + s : m0 + s + n],
                                start=True, stop=True,
                            )
                        od = o_tile[:, m0 - p0 : m0 - p0 + mw]
                        if pick == 0:
                            nc.vector.tensor_copy(od, ps[:, :mw])
                        else:
                            nc.scalar.activation(od, ps[:, :mw], copy_fn)
                    nc.gpsimd.dma_start(
                        out=yt[:, c0 + p0 : c0 + p0 + pw], in_=o_tile[:, :pw]
                    )
                c0 += cw

    nc.finalize()
    _PROGRAM_CACHE[key] = nc
    return nc


def _prepare(inputs):
    x = np.ascontiguousarray(inputs["edge_attr"], dtype=np.float32)

    Wv = inputs["Wv"].astype(np.float64)
    bv = inputs["bv"].astype(np.float64)
    W_in = inputs["W_in"].astype(np.float64)
    b_in = inputs["b_in"].astype(np.float64)
    Wiv = W_in[2 * HID : 3 * HID]
    biv = b_in[2 * HID : 3 * HID]
    W_mo = inputs["W_mo"].astype(np.float64)
    b_mo = inputs["b_mo"].astype(np.float64)
    Wo = inputs["Wo"].astype(np.float64)
    bo = inputs["bo"].astype(np.float64)

    M = 0.5 * (Wo @ W_mo @ Wiv @ Wv).T
    c = 0.5 * (((bv @ Wiv.T + biv) @ W_mo.T + b_mo) @ Wo.T + bo)

    if MODE == "bf16":
        wdev = np.ascontiguousarray(np.eye(HID) + M).astype(BF16)
        xdt = BF16
    else:
        wdev = np.ascontiguousarray(FP8_WSCALE * M).astype(FP8)
        xdt = FP8

    cf = c.astype(np.float32)

    nc = _build_program(MODE)

    in_maps = []
    for i in range(NCORES):
        shard = x[i * ESH : (i + 1) * ESH]
        in_maps.append(
            {"xt": np.ascontiguousarray(shard.T).astype(xdt), "wm": wdev}
        )

    return nc, in_maps, cf


def kernel(**inputs) -> np.ndarray:
    nc, in_maps, cf = _prepare(inputs)

    res = run_bass_kernel_spmd(nc, in_maps, list(range(NCORES)))

    out = np.empty((E, HID), dtype=np.float32)
    for i in range(NCORES):
        y = res.results[i]["yt"].astype(np.float32).T
        if MODE == "fp8":
            sh = inputs["edge_attr"][i * ESH : (i + 1) * ESH]
            out[i * ESH : (i + 1) * ESH] = sh + y * (1.0 / FP8_WSCALE)
        else:
            out[i * ESH : (i + 1) * ESH] = y
    if np.any(cf != 0.0):
        out += cf[None, :]
    return out
